# revision 1
# baseline (speedup 1.0000x reference)
"""Trainium2 Bass kernel for batched attention scores + softmax.

Computes, for hidden [1, B, H] and encoder_outputs [S, B, H]:
    scores[b, s] = dot(hidden[0, b, :], encoder_outputs[s, b, :])
    attn = softmax(scores, axis=-1)            -> returned as [B, 1, S]

Sharding: data-parallel over batch. B=64 is split across 8 NeuronCores
(8 batch elements per core); scores/softmax are independent per batch
element so there is no cross-core communication.

Per-core dataflow (all shapes per core):
  - hidden shard  [8, H]           -> SBUF once
  - for each b: broadcast hidden[b] to hb [128, H] via a K=1 PE matmul
    (ones-column stationary) + ScalarE PSUM->SBUF copies
  - encoder shard [S, 8, H] streams through SBUF in [128, 4, H] tiles
    (1 MiB per DMA, 4 KiB contiguous per descriptor), alternating between
    the two HWDGE rings; output/const DMAs ride SWDGE so their semaphore
    waits never stall the encoder stream.
  - one fused VectorE op (scalar_tensor_tensor with accumulate) per
    (b, s-chunk of 128): scratch = enc_tile * hb and
    scores[:, chunk] = sum_h in a single pass.
  - softmax over the [128, 16] per-b score tile:
        row max -> PE transpose -> global max -> exp(x - max) with
        accumulated sum on ScalarE -> total via ones-matmul -> DVE
        reciprocal -> PE transpose of exp -> normalize during the
        PSUM->SBUF copy -> DMA out.
"""

import numpy as np

import concourse.bass as bass
import concourse.bacc as bacc
import concourse.mybir as mybir
from concourse.tile import TileContext
from concourse.bass_utils import run_bass_kernel_spmd

F32 = mybir.dt.float32

# Problem geometry (hardcoded per the task contract).
S = 2048          # sequence length
B = 64            # total batch
H = 1024          # hidden size
N_CORES = 8
BSH = B // N_CORES  # batch elements per core
P = 128           # SBUF partitions / s-chunk size
NCH = S // P      # 16 s-chunks per batch element


def _load_groups(b: int) -> list[tuple[int, int]]:
    """(first_chunk, n_chunks) DMA groups for batch element b.

    1 MiB transfers for throughput; the very last batch element tapers to
    single-chunk loads so the final DMA->compute->softmax tail after the
    last transfer is short.
    """
    if b < BSH - 1:
        return [(0, 4), (4, 4), (8, 4), (12, 4)]
    return [(0, 4), (4, 4), (8, 4), (12, 2), (14, 1), (15, 1)]


def build_nc() -> bass.Bass:
    # Bacc (not raw Bass): its compile() pipeline splits multi-sem waits
    # (PE Matmult only supports one sync wait in walrus codegen).
    nc = bacc.Bacc("TRN2", target_bir_lowering=False, debug=False)

    hid_d = nc.declare_dram_parameter("hidden", [BSH, H], F32, isOutput=False)
    enc_d = nc.declare_dram_parameter("enc", [S, BSH, H], F32, isOutput=False)
    id_d = nc.declare_dram_parameter("ident", [P, P], F32, isOutput=False)
    out_d = nc.declare_dram_parameter("attn", [BSH, S], F32, isOutput=True)

    with TileContext(nc) as tc:
        with (
            tc.tile_pool(name="const", bufs=1) as constp,
            tc.tile_pool(name="encp", bufs=9) as encp,
            tc.tile_pool(name="hbp", bufs=2) as hbp,
            tc.tile_pool(name="scrp", bufs=3) as scrp,
            tc.tile_pool(name="smallp", bufs=2) as smallp,
            tc.tile_pool(name="ph_psum", bufs=1, space="PSUM") as ph_psum,
            tc.tile_pool(name="sm_psum", bufs=4, space="PSUM") as sm_psum,
        ):
            # const loads go through SWDGE (gpsimd) so the HWDGE rings'
            # first instructions are already encoder-tile streams
            ident = constp.tile([P, P], F32)
            nc.gpsimd.dma_start(out=ident[:], in_=id_d.ap())
            # single partition so any [1, 512] slice has base_partition 0
            # (PE matmul operands must start at partition 0/32/64)
            hid_sb = constp.tile([1, BSH * H], F32)
            nc.gpsimd.dma_start(out=hid_sb[:], in_=hid_d.ap().rearrange("b h -> (b h)"))

            ones_row = constp.tile([1, P], F32)
            nc.vector.memset(ones_row[:], 1.0)
            neg_row = constp.tile([1, P], F32)
            nc.vector.memset(neg_row[:], -1.0)
            ones_col = constp.tile([P, 1], F32)
            nc.vector.memset(ones_col[:], 1.0)

            enc_ap = enc_d.ap()
            out_ap = out_d.ap()
            dma_rr = [0]  # round-robin counter over the two HWDGE rings

            for b in range(BSH):
                # hb[p, h] = hidden[b, h] for every partition p.
                ph = ph_psum.tile([P, H], F32, tag="ph")
                nc.tensor.matmul(ph[:, 0:512], ones_row[:],
                                 hid_sb[0:1, b * H : b * H + 512],
                                 start=True, stop=True)
                nc.tensor.matmul(ph[:, 512:1024], ones_row[:],
                                 hid_sb[0:1, b * H + 512 : b * H + 1024],
                                 start=True, stop=True)
                hb = hbp.tile([P, H], F32, tag="hb")
                nc.scalar.copy(hb[:, 0:512], ph[:, 0:512])
                nc.scalar.copy(hb[:, 512:1024], ph[:, 512:1024])

                scores = smallp.tile([P, NCH], F32, tag="scores")
                for c0, glen in _load_groups(b):
                    et = encp.tile([P, glen, H], F32, tag="et")
                    src = enc_ap[c0 * P : (c0 + glen) * P, b, :].rearrange(
                        "(c p) h -> p c h", p=P
                    )
                    # alternate between the two HWDGE rings (SP and ACT)
                    dma_eng = nc.sync if dma_rr[0] % 2 == 0 else nc.scalar
                    dma_rr[0] += 1
                    dma_eng.dma_start(out=et[:], in_=src)
                    for c in range(glen):
                        chunk = c0 + c
                        # fused multiply + H-reduction in one VectorE pass:
                        # scr = (et bypass 1.0) * hb ; scores[:,chunk] = sum(scr)
                        # (TensorScalarPtr with accumulate — standard ISA; the
                        # DVE tensor_tensor_reduce ucode op is not executable
                        # in this runtime environment.)
                        scr = scrp.tile([P, H], F32, tag="scr")
                        nc.vector.scalar_tensor_tensor(
                            out=scr[:], in0=et[:, c, :], scalar=1.0, in1=hb[:],
                            op0=mybir.AluOpType.bypass,
                            op1=mybir.AluOpType.mult,
                            accum_out=scores[:, chunk : chunk + 1],
                        )

                # ---- softmax over the 2048 scores of batch element b ----
                rowmax = smallp.tile([P, 1], F32, tag="rowmax")
                nc.vector.reduce_max(rowmax[:], scores[:], axis=mybir.AxisListType.X)
                pmaxt = sm_psum.tile([1, P], F32, tag="sp")
                nc.tensor.transpose(pmaxt[:], rowmax[:], ident[:])
                gmax = smallp.tile([1, 1], F32, tag="gmax")
                nc.vector.reduce_max(gmax[:], pmaxt[:], axis=mybir.AxisListType.X)
                # -gmax broadcast to all 128 partitions (K=1 matmul with -1s)
                pneg = sm_psum.tile([P, 1], F32, tag="sp")
                nc.tensor.matmul(pneg[:], neg_row[:], gmax[:], start=True, stop=True)
                negb = smallp.tile([P, 1], F32, tag="negb")
                nc.scalar.copy(negb[:], pneg[:])

                expb = smallp.tile([P, NCH], F32, tag="expb")
                esum = smallp.tile([P, 1], F32, tag="esum")
                nc.scalar.activation(
                    expb[:], scores[:], mybir.ActivationFunctionType.Exp,
                    bias=negb[:], scale=1.0, accum_out=esum[:],
                )
                # transpose exp values immediately (runs on PE concurrently
                # with the sum/reciprocal chain below); [s_in_chunk, chunk]
                # -> [chunk, s_in_chunk] so the output DMA writes 512 B
                # contiguous runs.
                pattnt = sm_psum.tile([NCH, P], F32, tag="sp")
                nc.tensor.transpose(pattnt[:], expb[:], ident[:])

                # total = sum over partitions of esum (ones-matmul), then 1/total
                ptot = sm_psum.tile([1, 1], F32, tag="sp")
                nc.tensor.matmul(ptot[:], esum[:], ones_col[:], start=True, stop=True)
                rinv = smallp.tile([1, 1], F32, tag="rinv")
                nc.vector.reciprocal(rinv[:], ptot[:])
                prb = sm_psum.tile([NCH, 1], F32, tag="sp")
                nc.tensor.matmul(prb[:], ones_row[:, 0:NCH], rinv[:],
                                 start=True, stop=True)
                rinv16 = smallp.tile([NCH, 1], F32, tag="rinv16")
                nc.scalar.copy(rinv16[:], prb[:])

                # normalize during the PSUM->SBUF copy (per-partition scale)
                attnt = smallp.tile([NCH, P], F32, tag="attnt")
                nc.scalar.activation(
                    attnt[:], pattnt[:], mybir.ActivationFunctionType.Copy,
                    bias=0.0, scale=rinv16[:],
                )
                # SWDGE (gpsimd) so this DMA's wait on the epilogue never
                # blocks the HWDGE FIFOs that stream encoder tiles; the last
                # batch element has nothing queued behind it, so use the
                # lower-latency HWDGE ring there.
                out_eng = nc.sync if b == BSH - 1 else nc.gpsimd
                out_eng.dma_start(
                    out=out_ap[b, :].rearrange("(c p) -> c p", p=P),
                    in_=attnt[:],
                )

    return nc


def _in_maps(hidden: np.ndarray, encoder_outputs: np.ndarray) -> list[dict]:
    hidden = np.asarray(hidden, dtype=np.float32)
    encoder_outputs = np.asarray(encoder_outputs, dtype=np.float32)
    ident = np.eye(P, dtype=np.float32)
    maps = []
    for i in range(N_CORES):
        sl = slice(i * BSH, (i + 1) * BSH)
        maps.append(
            {
                "hidden": np.ascontiguousarray(hidden[0, sl, :]),
                "enc": np.ascontiguousarray(encoder_outputs[:, sl, :]),
                "ident": ident,
            }
        )
    return maps


def _run(in_maps: list[dict], **kwargs):
    nc = build_nc()
    # Bacc defers register allocation to finalize(); the axon/PJRT path
    # serializes the module as-is, so finalize must happen here.
    nc.finalize()
    return run_bass_kernel_spmd(nc, in_maps, list(range(N_CORES)), **kwargs)


def kernel(hidden: np.ndarray, encoder_outputs: np.ndarray) -> np.ndarray:
    res = _run(_in_maps(hidden, encoder_outputs))
    attn = np.concatenate([res.results[i]["attn"] for i in range(N_CORES)], axis=0)
    return attn[:, None, :].astype(np.float32)



# revision 2
# speedup vs baseline: 1.1614x; 1.1614x over previous
"""Trainium2 Bass kernel for batched attention scores + softmax.

Computes, for hidden [1, B, H] and encoder_outputs [S, B, H]:
    scores[b, s] = dot(hidden[0, b, :], encoder_outputs[s, b, :])
    attn = softmax(scores, axis=-1)            -> returned as [B, 1, S]

Sharding: data-parallel over batch. B=64 is split across 8 NeuronCores
(8 batch elements per core); scores/softmax are independent per batch
element so there is no cross-core communication.

v2 design (TensorEngine dot products + fully contiguous DMA):
  - Host relayouts the per-core encoder shard to [b, g, p, hk, s]:
    partition dim = h%128, so every [128, 4096] SBUF tile is ONE
    fully-contiguous 2 MiB DRAM block (16 KiB per partition, one
    descriptor per partition). 32 such tiles per core stream through
    the two HWDGE rings back-to-back at the HBM line rate.
  - Dot products run on the (otherwise idle) TensorEngine as float32r
    matmuls: scores[1, 512] += hidT[128h, 1].T @ enc[128h, 512s],
    accumulated over the 8 h-chunks of one tile into a [1, 512] PSUM
    group. float32r streams at 1 cycle/row (full bf16 speed) for
    moving dims >= 256, with fp32 PSUM accumulation. This removes the
    old VectorE multiply+reduce (181 us busy) from the critical path.
  - Online softmax per batch element, all on partition 0: per-group
    running max m_g and sum s_g = sum exp(x - m_g) are computed as each
    group's scores finish; the final pass rescales each group by
    exp(m_g - M) / sum_j s_j exp(m_j - M). Keeps the post-last-DMA
    tail to ~7 us instead of a full softmax over 2048 values.
  - Outputs ride SWDGE (gpsimd) so their semaphore waits never stall
    the HWDGE encoder stream; the last batch element uses the
    lower-latency HWDGE ring since nothing is queued behind it.
"""

import numpy as np

import concourse.bass as bass
import concourse.bacc as bacc
import concourse.mybir as mybir
from concourse.tile import TileContext
from concourse.bass_utils import run_bass_kernel_spmd

F32 = mybir.dt.float32
F32R = mybir.dt.float32r
Exp = mybir.ActivationFunctionType.Exp
Copy = mybir.ActivationFunctionType.Copy

# Problem geometry (hardcoded per the task contract).
S = 2048          # sequence length
B = 64            # total batch
H = 1024          # hidden size
N_CORES = 8
BSH = B // N_CORES  # batch elements per core
P = 128           # SBUF partitions
NG = 4            # score groups per batch element
GS = S // NG      # 512 scores per group (= one PSUM bank row)
NHK = H // P      # 8 h-chunks of 128
TFREE = NHK * GS  # 4096 f32 per partition per tile (16 KiB)


def build_nc() -> bass.Bass:
    # Bacc (not raw Bass): its compile() pipeline splits multi-sem waits
    # (PE Matmult only supports one sync wait in walrus codegen).
    nc = bacc.Bacc("TRN2", target_bir_lowering=False, debug=False)

    hid_d = nc.declare_dram_parameter("hid", [P, NHK * BSH], F32R, isOutput=False)
    enc_d = nc.declare_dram_parameter("enc", [BSH, NG, P, TFREE], F32R, isOutput=False)
    out_d = nc.declare_dram_parameter("attn", [BSH, S], F32, isOutput=True)

    with TileContext(nc) as tc:
        with (
            tc.tile_pool(name="const", bufs=1) as constp,
            tc.tile_pool(name="encp", bufs=6) as encp,
            tc.tile_pool(name="smallp", bufs=2) as smallp,
            tc.tile_pool(name="ps_pool", bufs=8, space="PSUM") as psp,
        ):
            # hidT[p, hk*BSH + b] = hidden[b, hk*128 + p]; SWDGE so the
            # HWDGE rings' first instructions are already encoder tiles.
            hid_sb = constp.tile([P, NHK * BSH], F32R)
            nc.gpsimd.dma_start(out=hid_sb[:], in_=hid_d.ap())

            enc_ap = enc_d.ap()
            out_ap = out_d.ap()
            rr = 0  # round-robin over the two HWDGE rings

            for b in range(BSH):
                maxcat = smallp.tile([1, NG], F32, tag="maxcat")
                sumcat = smallp.tile([1, NG], F32, tag="sumcat")
                expb = smallp.tile([1, S], F32, tag="expb")

                for g in range(NG):
                    et = encp.tile([P, TFREE], F32R, tag="et")
                    eng = nc.sync if rr % 2 == 0 else nc.scalar
                    rr += 1
                    eng.dma_start(out=et[:], in_=enc_ap[b, g])

                    # scores[0, s] = sum_h hid[b, h] * enc[g*512+s, b, h]
                    ps = psp.tile([1, GS], F32, tag="ps")
                    for hk in range(NHK):
                        col = hk * BSH + b
                        nc.tensor.matmul(
                            ps[:], hid_sb[:, col : col + 1],
                            et[:, hk * GS : (hk + 1) * GS],
                            start=(hk == 0), stop=(hk == NHK - 1),
                        )

                    # online softmax pieces: m_g, then e_g = exp(x - m_g)
                    # and s_g = sum(e_g), streamed while later tiles load.
                    nc.vector.reduce_max(
                        maxcat[:, g : g + 1], ps[:], axis=mybir.AxisListType.X
                    )
                    ngm = smallp.tile([1, 1], F32, tag="ngm")
                    nc.scalar.activation(
                        ngm[:], maxcat[:, g : g + 1], Copy, bias=0.0, scale=-1.0
                    )
                    nc.scalar.activation(
                        expb[:, g * GS : (g + 1) * GS], ps[:], Exp,
                        bias=ngm[:], scale=1.0,
                        accum_out=sumcat[:, g : g + 1],
                    )

                # combine groups: M = max_g m_g, D = sum_g s_g * exp(m_g - M),
                # per-group output scale = exp(m_g - M) / D.
                gmax = smallp.tile([1, 1], F32, tag="gmax")
                nc.vector.reduce_max(gmax[:], maxcat[:], axis=mybir.AxisListType.X)
                ngmax = smallp.tile([1, 1], F32, tag="ngmax")
                nc.scalar.activation(ngmax[:], gmax[:], Copy, bias=0.0, scale=-1.0)
                em = smallp.tile([1, NG], F32, tag="em")
                nc.scalar.activation(em[:], maxcat[:], Exp, bias=ngmax[:], scale=1.0)
                djunk = smallp.tile([1, NG], F32, tag="djunk")
                dsum = smallp.tile([1, 1], F32, tag="dsum")
                nc.vector.scalar_tensor_tensor(
                    out=djunk[:], in0=sumcat[:], scalar=1.0, in1=em[:],
                    op0=mybir.AluOpType.bypass,
                    op1=mybir.AluOpType.mult,
                    accum_out=dsum[:],
                )
                rinv = smallp.tile([1, 1], F32, tag="rinv")
                nc.vector.reciprocal(rinv[:], dsum[:])
                scl = smallp.tile([1, NG], F32, tag="scl")
                nc.scalar.activation(scl[:], em[:], Copy, bias=0.0, scale=rinv[:])

                attn_sb = smallp.tile([1, S], F32, tag="attn_sb")
                for g in range(NG):
                    nc.scalar.activation(
                        attn_sb[:, g * GS : (g + 1) * GS],
                        expb[:, g * GS : (g + 1) * GS],
                        Copy, bias=0.0, scale=scl[:, g : g + 1],
                    )
                # SWDGE so this DMA's wait on the softmax never blocks the
                # HWDGE FIFOs streaming encoder tiles; the last batch
                # element has nothing queued behind it -> low-latency HWDGE.
                out_eng = nc.sync if b == BSH - 1 else nc.gpsimd
                out_eng.dma_start(out=out_ap[b : b + 1, :], in_=attn_sb[:])

    return nc


def _in_maps(hidden: np.ndarray, encoder_outputs: np.ndarray) -> list[dict]:
    hidden = np.asarray(hidden, dtype=np.float32)
    encoder_outputs = np.asarray(encoder_outputs, dtype=np.float32)
    maps = []
    for i in range(N_CORES):
        sl = slice(i * BSH, (i + 1) * BSH)
        h_core = hidden[0, sl, :]                      # [BSH, H]
        hid_t = np.ascontiguousarray(
            h_core.reshape(BSH, NHK, P).transpose(2, 1, 0)
        ).reshape(P, NHK * BSH)                        # [p, hk, b]
        e_core = encoder_outputs[:, sl, :]             # [S, BSH, H]
        e5 = e_core.reshape(NG, GS, BSH, NHK, P)       # [g, s, b, hk, p]
        enc_t = np.ascontiguousarray(
            e5.transpose(2, 0, 4, 3, 1)                # [b, g, p, hk, s]
        ).reshape(BSH, NG, P, TFREE)
        maps.append({"hid": hid_t, "enc": enc_t})
    return maps


def _run(in_maps: list[dict], **kwargs):
    nc = build_nc()
    # Bacc defers register allocation to finalize(); the axon/PJRT path
    # serializes the module as-is, so finalize must happen here.
    nc.finalize()
    return run_bass_kernel_spmd(nc, in_maps, list(range(N_CORES)), **kwargs)


def kernel(hidden: np.ndarray, encoder_outputs: np.ndarray) -> np.ndarray:
    res = _run(_in_maps(hidden, encoder_outputs))
    attn = np.concatenate([res.results[i]["attn"] for i in range(N_CORES)], axis=0)
    return attn[:, None, :].astype(np.float32)


# revision 4
# speedup vs baseline: 1.2349x; 1.0633x over previous
"""Trainium2 Bass kernel for batched attention scores + softmax.

Computes, for hidden [1, B, H] and encoder_outputs [S, B, H]:
    scores[b, s] = dot(hidden[0, b, :], encoder_outputs[s, b, :])
    attn = softmax(scores, axis=-1)            -> returned as [B, 1, S]

Sharding: data-parallel over batch. B=64 is split across 8 NeuronCores
(8 batch elements per core); scores/softmax are independent per batch
element so there is no cross-core communication.

v2 design (TensorEngine dot products + fully contiguous DMA):
  - Host relayouts the per-core encoder shard to [b, g, p, hk, s]:
    partition dim = h%128, so every [128, 4096] SBUF tile is ONE
    fully-contiguous 2 MiB DRAM block (16 KiB per partition, one
    descriptor per partition). 32 such tiles per core stream through
    the two HWDGE rings back-to-back at the HBM line rate.
  - Dot products run on the (otherwise idle) TensorEngine as float32r
    matmuls: scores[1, 512] += hidT[128h, 1].T @ enc[128h, 512s],
    accumulated over the 8 h-chunks of one tile into a [1, 512] PSUM
    group. float32r streams at 1 cycle/row (full bf16 speed) for
    moving dims >= 256, with fp32 PSUM accumulation. This removes the
    old VectorE multiply+reduce (181 us busy) from the critical path.
  - Online softmax per batch element, all on partition 0: per-group
    running max m_g and sum s_g = sum exp(x - m_g) are computed as each
    group's scores finish; the final pass rescales each group by
    exp(m_g - M) / sum_j s_j exp(m_j - M). Keeps the post-last-DMA
    tail to ~7 us instead of a full softmax over 2048 values.
  - Outputs ride SWDGE (gpsimd) so their semaphore waits never stall
    the HWDGE encoder stream; the last batch element uses the
    lower-latency HWDGE ring since nothing is queued behind it.
"""

import numpy as np

import concourse.bass as bass
import concourse.bacc as bacc
import concourse.mybir as mybir
from concourse.tile import TileContext
from concourse.bass_utils import run_bass_kernel_spmd

F32 = mybir.dt.float32
F32R = mybir.dt.float32r
Exp = mybir.ActivationFunctionType.Exp
Copy = mybir.ActivationFunctionType.Copy

# Problem geometry (hardcoded per the task contract).
S = 2048          # sequence length
B = 64            # total batch
H = 1024          # hidden size
N_CORES = 8
BSH = B // N_CORES  # batch elements per core
P = 128           # SBUF partitions
NG = 4            # score groups per batch element
GS = S // NG      # 512 scores per group (= one PSUM bank row)
NHK = H // P      # 8 h-chunks of 128
TFREE = NHK * GS  # 4096 f32 per partition per tile (16 KiB)


def build_nc() -> bass.Bass:
    # Bacc (not raw Bass): its compile() pipeline splits multi-sem waits
    # (PE Matmult only supports one sync wait in walrus codegen).
    nc = bacc.Bacc("TRN2", target_bir_lowering=False, debug=False)

    hid_d = nc.declare_dram_parameter("hid", [P, NHK * BSH], F32R, isOutput=False)
    enc_d = nc.declare_dram_parameter("enc", [BSH, NG, P, TFREE], F32R, isOutput=False)
    out_d = nc.declare_dram_parameter("attn", [BSH, S], F32, isOutput=True)

    with TileContext(nc) as tc:
        with (
            tc.tile_pool(name="const", bufs=1) as constp,
            tc.tile_pool(name="encp", bufs=7) as encp,
            tc.tile_pool(name="smallp", bufs=2) as smallp,
            tc.tile_pool(name="ps_pool", bufs=8, space="PSUM") as psp,
        ):
            # hidT[p, hk*BSH + b] = hidden[b, hk*128 + p]; SWDGE so the
            # HWDGE ring's first instructions are already encoder tiles.
            hid_sb = constp.tile([P, NHK * BSH], F32R)
            nc.gpsimd.dma_start(out=hid_sb[:], in_=hid_d.ap())

            enc_ap = enc_d.ap()
            out_ap = out_d.ap()

            for b in range(BSH):
                # mnegcat[g] = -m_g (reduce negate=True), so the exp bias
                # needs no separate negation op on ScalarE.
                mnegcat = smallp.tile([1, NG], F32, tag="mnegcat")
                sumcat = smallp.tile([1, NG], F32, tag="sumcat")
                expb = smallp.tile([1, S], F32, tag="expb")

                for g in range(NG):
                    et = encp.tile([P, TFREE], F32R, tag="et")
                    # ALL encoder tiles on the SP HWDGE ring: the ACT ring's
                    # sequencer also runs the softmax activations, so a DMA
                    # issued there queues behind compute waits and starves
                    # the stream (measured 16 us of mid-stream DMA idle).
                    nc.sync.dma_start(out=et[:], in_=enc_ap[b, g])

                    # scores[0, s] = sum_h hid[b, h] * enc[g*512+s, b, h]
                    ps = psp.tile([1, GS], F32, tag="ps")
                    for hk in range(NHK):
                        col = hk * BSH + b
                        nc.tensor.matmul(
                            ps[:], hid_sb[:, col : col + 1],
                            et[:, hk * GS : (hk + 1) * GS],
                            start=(hk == 0), stop=(hk == NHK - 1),
                        )

                    # online softmax pieces: -m_g, then e_g = exp(x - m_g)
                    # and s_g = sum(e_g), streamed while later tiles load.
                    nc.vector.reduce_max(
                        mnegcat[:, g : g + 1], ps[:], axis=mybir.AxisListType.X,
                        negate=True,
                    )
                    nc.scalar.activation(
                        expb[:, g * GS : (g + 1) * GS], ps[:], Exp,
                        bias=mnegcat[:, g : g + 1], scale=1.0,
                        accum_out=sumcat[:, g : g + 1],
                    )

                # combine groups: M = max_g m_g, D = sum_g s_g * exp(m_g - M),
                # per-group output scale = exp(m_g - M) / D.
                # -M = min_g(-m_g); em_g = exp(-1 * mneg_g + (-M)).
                mneg = smallp.tile([1, 1], F32, tag="mneg")
                nc.vector.tensor_reduce(
                    mneg[:], mnegcat[:], axis=mybir.AxisListType.X,
                    op=mybir.AluOpType.min,
                )
                em = smallp.tile([1, NG], F32, tag="em")
                nc.scalar.activation(em[:], mnegcat[:], Exp, bias=mneg[:], scale=-1.0)
                djunk = smallp.tile([1, NG], F32, tag="djunk")
                dsum = smallp.tile([1, 1], F32, tag="dsum")
                nc.vector.scalar_tensor_tensor(
                    out=djunk[:], in0=sumcat[:], scalar=1.0, in1=em[:],
                    op0=mybir.AluOpType.bypass,
                    op1=mybir.AluOpType.mult,
                    accum_out=dsum[:],
                )
                rinv = smallp.tile([1, 1], F32, tag="rinv")
                nc.vector.reciprocal(rinv[:], dsum[:])
                scl = smallp.tile([1, NG], F32, tag="scl")
                nc.scalar.activation(scl[:], em[:], Copy, bias=0.0, scale=rinv[:])

                attn_sb = smallp.tile([1, S], F32, tag="attn_sb")
                for g in range(NG):
                    nc.scalar.activation(
                        attn_sb[:, g * GS : (g + 1) * GS],
                        expb[:, g * GS : (g + 1) * GS],
                        Copy, bias=0.0, scale=scl[:, g : g + 1],
                    )
                # SWDGE so this DMA's wait on the softmax never blocks the
                # SP HWDGE FIFO streaming encoder tiles; the last batch
                # element rides the (by then idle) ACT HWDGE ring for its
                # lower first-byte latency.
                out_eng = nc.scalar if b == BSH - 1 else nc.gpsimd
                out_eng.dma_start(out=out_ap[b : b + 1, :], in_=attn_sb[:])

    return nc


def _in_maps(hidden: np.ndarray, encoder_outputs: np.ndarray) -> list[dict]:
    hidden = np.asarray(hidden, dtype=np.float32)
    encoder_outputs = np.asarray(encoder_outputs, dtype=np.float32)
    maps = []
    for i in range(N_CORES):
        sl = slice(i * BSH, (i + 1) * BSH)
        h_core = hidden[0, sl, :]                      # [BSH, H]
        hid_t = np.ascontiguousarray(
            h_core.reshape(BSH, NHK, P).transpose(2, 1, 0)
        ).reshape(P, NHK * BSH)                        # [p, hk, b]
        e_core = encoder_outputs[:, sl, :]             # [S, BSH, H]
        e5 = e_core.reshape(NG, GS, BSH, NHK, P)       # [g, s, b, hk, p]
        enc_t = np.ascontiguousarray(
            e5.transpose(2, 0, 4, 3, 1)                # [b, g, p, hk, s]
        ).reshape(BSH, NG, P, TFREE)
        maps.append({"hid": hid_t, "enc": enc_t})
    return maps


def _run(in_maps: list[dict], **kwargs):
    nc = build_nc()
    # Bacc defers register allocation to finalize(); the axon/PJRT path
    # serializes the module as-is, so finalize must happen here.
    nc.finalize()
    return run_bass_kernel_spmd(nc, in_maps, list(range(N_CORES)), **kwargs)


def kernel(hidden: np.ndarray, encoder_outputs: np.ndarray) -> np.ndarray:
    res = _run(_in_maps(hidden, encoder_outputs))
    attn = np.concatenate([res.results[i]["attn"] for i in range(N_CORES)], axis=0)
    return attn[:, None, :].astype(np.float32)
